# revision 29
# baseline (speedup 1.0000x reference)
"""Trainium2 Bass kernel for AugmentedGraphNeuralODEFunc.

Reference computation (B=4, N=512, AUG=32, ORIG=16, HID=128):
  edge_features[b,i,j] = [z_i(32), z_j(32), p_i-p_j(3), |p_i-p_j|(1),
                          ps_i-ps_j(3), |ps_i-ps_j|(1)]       (72)
  msg = MLP(72->128->128->16) per edge; agg_i = sum_j msg_ij
  d_evolving = MLP(32->128->128->16)([z_i[:16], agg_i]); static half -> 0

Algebraic restructure used on device:
  layer1 pre-act for receiver i, sender j:
    h1[:,j] = W_B^T z_j + A_i + dist_ij * v + dist_s_ij * w
  where A_i = W_A^T z_i + eb0 (diff terms fold into W_A/W_B since
  diff = p_i - p_j is linear in z), v/w are the dist rows of eW0.
  dist^2 via the Gram identity r_i + r_j - 2 p_i.p_j as one K=16 matmul
  (hi/lo bf16 split for near-fp32 accuracy), then clamp + sqrt.
  Layer 3 + bias commute with the sum over j:
    agg_i = (sum_j relu(h2_ij)) @ eW2 + N*eb2.

Sharding: receivers (dim 1 of the NxN edge tensor) split across 8 cores,
64 receivers x 4 batches = 256 receiver-pairs per core; the sum over
senders is local so there is no cross-core communication.

Layer-1 matmul per pair, K padded to 128 with zero rows (the TRN2 HAM
clock-gate never warms to 2.4 GHz under K<128 streams).  Stationary
operand shared across 16 pairs of a (b, a) group:
  LGALL[:, g=4b+a, :] rows: W_B(0:32) | A_{b,16a+s} at 32+s | v at
  48+2a | w at 49+2a | 0.
  RVALL[:, b, s, :] rows: z_b.T(0:32) | slot-indicator rows 32:48 |
  dist lanes 48:56 (lane 48+2a+h holds dist_h of pair 16a+s) | 0.
A-rows, indicators and all weight folds are host-prepared (O(N*HID));
only the O(N^2) distance/edge work runs on device.

Main loop per pair: mm_f (PE, 512 cols) -> relu1 (ACT, batched 2 pairs)
-> mm2 (PE) -> relu2+j-sum via accum_out (mostly DVE tensor_scalar, 1
in 8 on ACT to balance engine load).
Tail (256 pairs at once): agg = eW2^T S + N*eb2; update MLP.
"""

import ml_dtypes
import numpy as np

import concourse.bass as bass
import concourse.tile as tile
from concourse import bacc, mybir
from concourse.bass_utils import run_bass_kernel_spmd

ORIG = 16
AUG = 32
HID = 128
B = 4
N = 512
NCORES = 8
RECV = N // NCORES          # 64 receivers per core
PAIRS = B * RECV            # 256 (b, i) pairs per core
SLOTS = 16                  # pairs per stationary group
NGRP = PAIRS // SLOTS       # 16 stationary groups (4b + a)
GK = 16                     # Gram contraction rows (13 used)

F32 = mybir.dt.float32
BF16 = mybir.dt.bfloat16
AluOp = mybir.AluOpType
Act = mybir.ActivationFunctionType

# packed fp32 constant columns
_C_EB1 = 0
_C_UW1 = 1
_C_UB1 = 129
_C_EW2 = 130
_C_UW0 = 146
_C_UB0 = 274
_C_UW2 = 275
_C_NB2 = 291
_C_UB2 = 292
_C_UW0H = 293     # uW0[16:32] on partitions 0:16 (split-K update mm)
_C_NPK = 421

_PROGRAM_CACHE = {}


def build_program(act_every=7, e1b=2, pipe=4):
    nc = bacc.Bacc("TRN2", target_bir_lowering=False, debug=False)

    def din(name, shape, dt=F32):
        return nc.dram_tensor(name, shape, dt, kind="ExternalInput")

    lgall_d = din("lgall", [128, NGRP, HID], BF16)
    zbc_d = din("zbc", [B, AUG, SLOTS, N], BF16)
    ind_d = din("ind", [SLOTS, SLOTS, N], BF16)
    glall_d = din("glall", [GK, 2 * B * RECV], BF16)
    grall_d = din("grall", [GK, 2 * B * N], BF16)
    zpad_d = din("zpad", [72, SLOTS, N], BF16)
    ew1_d = din("ew1", [HID, HID], BF16)
    pk_d = din("pk", [128, _C_NPK])
    zrev_d = din("zrev", [ORIG, PAIRS])
    out_d = nc.dram_tensor("out", [ORIG, PAIRS], F32, kind="ExternalOutput")

    with tile.TileContext(nc) as tc:
        with (
            tc.tile_pool(name="const", bufs=1) as cp,
            tc.tile_pool(name="work", bufs=2) as wp,
            tc.tile_pool(name="ps0", bufs=2, space=bass.MemorySpace.PSUM) as pp0,
            tc.tile_pool(name="ps1", bufs=4, space=bass.MemorySpace.PSUM) as pp1,
        ):
            # ---------------- constants / staged inputs ----------------
            PK = cp.tile([128, _C_NPK], F32, name="pk")
            EW1 = cp.tile([HID, HID], BF16, name="ew1")
            ZREV = cp.tile([ORIG, PAIRS], F32, name="zrev")
            LGALL = cp.tile([128, NGRP, HID], BF16, name="lgall")
            GLALL = cp.tile([GK, 2 * B * RECV], BF16, name="glall")
            GRALL = cp.tile([GK, 2 * B * N], BF16, name="grall")
            nc.sync.dma_start(GLALL[:], glall_d[:])
            nc.scalar.dma_start(GRALL[:], grall_d[:])

            EB1 = PK[:, _C_EB1:_C_EB1 + 1]

            # moving operand: RVALL[:, b, s, :] is one K=128 column stack.
            # Rows 48:128 zero-filled by DMA (no partition-alignment rule,
            # keeps the vector engines free); dist lanes overwrite 48:56.
            # Staging is strictly per-batch so b=0 is ready ASAP and the
            # main loop overlaps b=1..3 staging; the small dist-lane DMAs
            # issue from the idle GpSimd queue so they never queue behind
            # the fat staging transfers on Sync.
            # b=0 staging spread across three DGE queues; the fat b>=1
            # staging is issued from inside the main loop (the Sync queue
            # is idle there) so it never delays b0's small dist-lane DMAs
            # in the DMA engines.
            RVALL = cp.tile([128, B, SLOTS, N], BF16, name="rvall")

            def stage_fat(b):
                nc.sync.dma_start(RVALL[0:AUG, b, :, :], zbc_d[b])
                nc.sync.dma_start(RVALL[AUG:48, b, :, :], ind_d[:])
                nc.sync.dma_start(RVALL[56:128, b, :, :], zpad_d[:])

            for b in range(B):
                if b == 0:
                    nc.sync.dma_start(RVALL[0:AUG, b, :, :], zbc_d[b])
                    nc.scalar.dma_start(RVALL[AUG:48, b, :, :], ind_d[:])
                    nc.gpsimd.dma_start(RVALL[56:128, b, :, :], zpad_d[:])
                    nc.sync.dma_start(LGALL[:], lgall_d[:])
                    nc.scalar.dma_start(PK[:], pk_d[:])
                    nc.gpsimd.dma_start(EW1[:], ew1_d[:])
                    nc.scalar.dma_start(ZREV[:], zrev_d[:])
                for h in range(2):
                    blk = h * B + b
                    g_ps = pp1.tile([RECV, N], F32, tag="psum1", name="g_ps")
                    nc.tensor.matmul(
                        g_ps[:],
                        GLALL[:, RECV * blk:RECV * (blk + 1)],
                        GRALL[:, N * blk:N * (blk + 1)],
                        start=True, stop=True,
                    )
                    d2 = wp.tile([RECV, N], F32, tag="d2", name="d2",
                                 bufs=4)
                    nc.vector.tensor_scalar(
                        out=d2[:], in0=g_ps[:],
                        scalar1=0.0, scalar2=None, op0=AluOp.max,
                    )
                    dsq = wp.tile([RECV, N], BF16, tag="dsq", name="dsq",
                                  bufs=4)
                    nc.scalar.sqrt(dsq[:], d2[:])
                    for a in range(4):
                        lane = 48 + 2 * a + h
                        nc.gpsimd.dma_start(
                            RVALL[lane:lane + 1, b, :, :],
                            dsq[SLOTS * a:SLOTS * (a + 1), :],
                        )

            ZER = cp.tile([HID, N], BF16, name="zer")
            nc.vector.memset(ZER[:], 0.0)

            S = cp.tile([HID, PAIRS], F32, name="s_acc")

            h1s = {}

            def emit_front(idx):
                b, p = idx // RECV, idx % RECV
                a, s = p // SLOTS, p % SLOTS
                g, lane = idx // e1b, idx % e1b
                if lane == 0:
                    emit_front.psum0 = pp0.tile(
                        [128, e1b * N], F32, tag="psum0", name="psum0")
                nc.tensor.matmul(
                    emit_front.psum0[:, N * lane:N * (lane + 1)],
                    LGALL[:, 4 * b + a, :], RVALL[:, b, s, :],
                    start=True, stop=True,
                )
                if lane == e1b - 1 or idx == PAIRS - 1:
                    nlan = lane + 1
                    h1 = wp.tile([128, e1b * N], BF16, tag="h1", name="h1",
                                 bufs=4)
                    nc.scalar.activation(
                        out=h1[:, 0:N * nlan],
                        in_=emit_front.psum0[:, 0:N * nlan], func=Act.Relu,
                    )
                    h1s[g] = h1

            def emit_back(q):
                g, lane = q // e1b, q % e1b
                h1 = h1s[g]
                psum1 = pp1.tile([HID, N], F32, tag="psum1", name="psum1")
                nc.tensor.matmul(
                    psum1[:], EW1[:], h1[:, N * lane:N * (lane + 1)],
                    start=True, stop=True,
                )
                h2s = wp.tile([HID, N], BF16, tag="h2s", name="h2s", bufs=4)
                if q % act_every == 0:
                    nc.scalar.activation(
                        out=h2s[:], in_=psum1[:],
                        func=Act.Relu, bias=EB1, scale=1.0,
                        accum_out=S[:, q:q + 1],
                    )
                else:
                    # NOTE: tensor_scalar with scalar1=AP + scalar2 imm +
                    # accum_out produces wrong results on HW (tested twice,
                    # identical bad output) — keep the STT form.
                    nc.vector.scalar_tensor_tensor(
                        out=h2s[:], in0=psum1[:],
                        scalar=EB1, in1=ZER[:],
                        op0=AluOp.add, op1=AluOp.max,
                        accum_out=S[:, q:q + 1],
                    )

            for idx in range(PAIRS + pipe):
                if idx == 8:
                    stage_fat(1)
                elif idx == 72:
                    stage_fat(2)
                elif idx == 136:
                    stage_fat(3)
                if idx < PAIRS:
                    emit_front(idx)
                if idx >= pipe:
                    emit_back(idx - pipe)

            # ---------------- tail: agg + update MLP -----------------
            # u1 = uW0[0:16]^T zrev + uW0[16:32]^T agg via a split-K
            # accumulating matmul pair (no U-assembly DMA needed).
            agg_ps = pp1.tile([ORIG, PAIRS], F32, tag="psum1", name="agg_ps")
            nc.tensor.matmul(
                agg_ps[:], PK[:, _C_EW2:_C_EW2 + ORIG], S[:],
                start=True, stop=True,
            )
            AGGSB = cp.tile([ORIG, PAIRS], F32, name="aggsb")
            nc.vector.tensor_scalar(
                out=AGGSB[:], in0=agg_ps[:],
                scalar1=PK[0:ORIG, _C_NB2:_C_NB2 + 1], scalar2=None,
                op0=AluOp.add,
            )

            u1_ps = pp1.tile([HID, PAIRS], F32, tag="psum1", name="u1_ps")
            nc.tensor.matmul(
                u1_ps[:], PK[0:ORIG, _C_UW0:_C_UW0 + HID], ZREV[:],
                start=True, stop=False,
            )
            nc.tensor.matmul(
                u1_ps[:], PK[0:ORIG, _C_UW0H:_C_UW0H + HID], AGGSB[:],
                start=False, stop=True,
            )
            HU1 = cp.tile([HID, PAIRS], F32, name="hu1")
            nc.scalar.activation(
                out=HU1[:], in_=u1_ps[:], func=Act.Relu,
                bias=PK[:, _C_UB0:_C_UB0 + 1], scale=1.0,
            )
            u2_ps = pp1.tile([HID, PAIRS], F32, tag="psum1", name="u2_ps")
            nc.tensor.matmul(
                u2_ps[:], PK[:, _C_UW1:_C_UW1 + HID], HU1[:],
                start=True, stop=True,
            )
            HU2 = cp.tile([HID, PAIRS], F32, name="hu2")
            nc.scalar.activation(
                out=HU2[:], in_=u2_ps[:], func=Act.Relu,
                bias=PK[:, _C_UB1:_C_UB1 + 1], scale=1.0,
            )
            u3_ps = pp1.tile([ORIG, PAIRS], F32, tag="psum1", name="u3_ps")
            nc.tensor.matmul(
                u3_ps[:], PK[:, _C_UW2:_C_UW2 + ORIG], HU2[:],
                start=True, stop=True,
            )
            OUTSB = cp.tile([ORIG, PAIRS], F32, name="outsb")
            nc.vector.tensor_scalar(
                out=OUTSB[:], in0=u3_ps[:],
                scalar1=PK[0:ORIG, _C_UB2:_C_UB2 + 1], scalar2=None,
                op0=AluOp.add,
            )
            nc.sync.dma_start(out_d[:], OUTSB[:])

    nc.compile()
    return nc


def _host_prep(z_aug, eW0, eb0, eW1, eb1, eW2, eb2,
               uW0, ub0, uW1, ub1, uW2, ub2):
    f = np.float32
    bf = ml_dtypes.bfloat16
    z = np.ascontiguousarray(z_aug, dtype=f)
    zt = z.transpose(0, 2, 1)                            # [B, 32, N]

    # Gram operands, bf16, hi/lo split so the fp32 PSUM accumulation
    # reconstructs near-fp32 distances:
    #   D2 = r_i + r_j - 2 [ph_i.ph_j + ph_i.pl_j + pl_i.ph_j]
    def hilo(x):
        hi = x.astype(bf).astype(f)
        lo = (x - hi).astype(bf).astype(f)
        return hi, lo

    gr = np.zeros((B, 2, GK, N), f)
    gl = np.zeros((B, 2, GK, N), f)
    for h, rows in enumerate([(0, 3), (16, 19)]):
        pfull = zt[:, rows[0]:rows[1], :]
        ph, pl = hilo(pfull)
        r = ((ph + pl) ** 2).sum(axis=1)
        rh, rl = hilo(r)
        gr[:, h, 0:3] = ph
        gr[:, h, 3:6] = pl
        gr[:, h, 6:9] = ph
        gr[:, h, 9] = 1.0
        gr[:, h, 10] = 1.0
        gr[:, h, 11] = rh
        gr[:, h, 12] = rl
        gl[:, h, 0:3] = -2.0 * ph
        gl[:, h, 3:6] = -2.0 * ph
        gl[:, h, 6:9] = -2.0 * pl
        gl[:, h, 9] = rh
        gl[:, h, 10] = rl
        gl[:, h, 11] = 1.0
        gl[:, h, 12] = 1.0

    eW0 = np.asarray(eW0, f)
    WA = eW0[0:32].copy()
    WA[0:3] += eW0[64:67]
    WA[16:19] += eW0[68:71]
    WB = eW0[32:64].copy()
    WB[0:3] -= eW0[64:67]
    WB[16:19] -= eW0[68:71]
    A_full = z @ WA + np.asarray(eb0, f)                 # [B, N, HID]

    ind = np.zeros((SLOTS, SLOTS, N), f)
    for s in range(SLOTS):
        ind[s, s, :] = 1.0

    pk = np.zeros((128, _C_NPK), f)
    pk[:, _C_EB1] = np.asarray(eb1, f)
    pk[:, _C_UW1:_C_UW1 + HID] = np.asarray(uW1, f)
    pk[:, _C_UB1] = np.asarray(ub1, f)
    pk[:, _C_EW2:_C_EW2 + ORIG] = np.asarray(eW2, f)
    pk[0:AUG, _C_UW0:_C_UW0 + HID] = np.asarray(uW0, f)
    pk[0:ORIG, _C_UW0H:_C_UW0H + HID] = np.asarray(uW0, f)[ORIG:AUG]
    pk[:, _C_UB0] = np.asarray(ub0, f)
    pk[:, _C_UW2:_C_UW2 + ORIG] = np.asarray(uW2, f)
    pk[0:ORIG, _C_NB2] = np.asarray(eb2, f) * np.float32(N)
    pk[0:ORIG, _C_UB2] = np.asarray(ub2, f)

    zbc = np.broadcast_to(
        zt.astype(bf)[:, :, None, :], (B, AUG, SLOTS, N))
    common = {
        "zbc": np.ascontiguousarray(zbc),
        "ind": ind.astype(bf),
        "zpad": np.zeros((72, SLOTS, N), bf),
        "ew1": np.ascontiguousarray(np.asarray(eW1, f)).astype(bf),
        "pk": pk,
    }
    in_maps = []
    for c in range(NCORES):
        sl = slice(RECV * c, RECV * (c + 1))
        lg = np.zeros((128, NGRP, HID), f)
        lg[0:AUG] = WB[:, None, :]
        for b in range(B):
            for a in range(4):
                g = 4 * b + a
                for s in range(SLOTS):
                    lg[AUG + s, g] = A_full[b, RECV * c + SLOTS * a + s]
                lg[48 + 2 * a, g] = eW0[67]
                lg[49 + 2 * a, g] = eW0[71]
        glc = gl[:, :, :, sl]                            # [B, 2, GK, RECV]
        glall = np.zeros((GK, 2 * B * RECV), f)
        grall = np.zeros((GK, 2 * B * N), f)
        for h in range(2):
            for b in range(B):
                blk = h * B + b
                glall[:, RECV * blk:RECV * (blk + 1)] = glc[b, h]
                grall[:, N * blk:N * (blk + 1)] = gr[b, h]
        zrev = np.zeros((ORIG, PAIRS), f)
        for b in range(B):
            zrev[:, RECV * b:RECV * (b + 1)] = zt[b, 0:ORIG, sl]
        m = dict(common)
        m["lgall"] = lg.astype(bf)
        m["glall"] = glall.astype(bf)
        m["grall"] = grall.astype(bf)
        m["zrev"] = zrev
        in_maps.append(m)
    return in_maps


def _assemble(results, dtype):
    out = np.zeros((B, N, AUG), dtype=dtype)
    for c in range(NCORES):
        o = results[c]["out"]                 # [ORIG, PAIRS]
        for b in range(B):
            out[b, RECV * c:RECV * (c + 1), 0:ORIG] = \
                o[:, RECV * b:RECV * (b + 1)].T
    return out


def run(inputs, trace=False, **trace_kwargs):
    key = "v2"
    if key not in _PROGRAM_CACHE:
        _PROGRAM_CACHE[key] = build_program()
    nc = _PROGRAM_CACHE[key]
    in_maps = _host_prep(
        inputs["z_aug"], inputs["eW0"], inputs["eb0"], inputs["eW1"],
        inputs["eb1"], inputs["eW2"], inputs["eb2"], inputs["uW0"],
        inputs["ub0"], inputs["uW1"], inputs["ub1"], inputs["uW2"],
        inputs["ub2"],
    )
    res = run_bass_kernel_spmd(
        nc, in_maps, list(range(NCORES)), trace=trace, **trace_kwargs
    )
    out = _assemble(res.results, np.asarray(inputs["z_aug"]).dtype)
    return out, res


def kernel(**inputs):
    out, _ = run(inputs, trace=False)
    return out
